# revision 6
# baseline (speedup 1.0000x reference)
"""Multi-head graph-attention (GAT) kernel for Trainium2, 8 NeuronCores.

Reference computation (per head):
    h_prime = h @ w[head]                       # [8192, 64]
    s = h_prime @ a_src[head],  d = h_prime @ a_dst[head]
    attn = softmax_j(leaky_relu(s_i + d_j, 0.2))
    out  = attn @ h_prime + bias                # -> [8192, 4*64]

Key identity: with exp monotone,
    exp(lrelu(s_i + d_j)) = e^{s_i} e^{d_j}           if s_i + d_j >= 0
                          = e^{0.2 s_i} e^{0.2 d_j}   otherwise
The mask sets {j : d_j >= -s_i} are NESTED across i, so the whole O(n^2)
attention contraction is a 1-D step function of t = -s_i:
    g(t) = sum_{j: d_j >= t} [v_j h'_j | v_j],   v = e^d   (and gq with q)
Evaluate g on a fixed grid of B=256 thresholds t_b (one fp16 matmul with
only B moving columns per 128-row j-tile), then each query i reads column
b_i = round((-s_i - LO)/G).  Grid rounding only misclassifies js with
|s_i + d_j| <= G, where exp(lrelu(x)) is continuous, so the error is
O(G^2) ~ 1e-4 relative.  Everything else is exact (d comparisons in full
precision; per-query exp factors exact):
    num_i / e^{s_i} = A(b_i) - r_i (C(b_i) - Sq),   r_i = e^{-0.8 s_i}
    den_i / e^{s_i} = a(b_i) - r_i (c(b_i) - sq)
with A|a = g_v columns, C|c = g_q columns, Sq|sq the full q-sums
(= g_q at the leftmost grid point).

Sharding: 8 cores = 4 heads x 2 query-halves. Each core gets the full h
(rows rotated so its query half is rows 0..4095 - j-side sums are order
invariant), computes the full j-side tables, and evaluates its 4096
queries. No collectives.
"""

import numpy as np

import concourse.bass as bass
import concourse.tile as tile
from concourse import bacc, mybir
from concourse.bass_utils import run_bass_kernel_spmd
from concourse.masks import make_identity

F32 = mybir.dt.float32
BF16 = mybir.dt.bfloat16
FP16 = mybir.dt.float16
I16 = mybir.dt.int16
I32 = mybir.dt.int32
AF = mybir.ActivationFunctionType
OP = mybir.AluOpType

BS = 8192          # nodes
F = 64             # f_in == f_out
NH = 4             # heads
HALF = BS // 2     # queries per core
NT_J = BS // 128   # 64 j tiles
NT_I = HALF // 128 # 32 query tiles
ALPHA = 0.2

BGRID = 256        # grid points
LO = -8.0          # grid range [LO, -LO)
G = (-2.0 * LO) / BGRID   # 1/16
INVG = 1.0 / G

AUGW = 2 * (F + 1)  # 130: [Hv|v | Hq|q] stationary width


def _build_kernel_module():
    nc = bacc.Bacc("TRN2", target_bir_lowering=False, debug=False)

    hfull_d = nc.dram_tensor("hfull", [BS, F], F32, kind="ExternalInput")
    w_d = nc.dram_tensor("w", [F, F], F32, kind="ExternalInput")
    aa_d = nc.dram_tensor("aa", [F, 2], F32, kind="ExternalInput")
    bias_d = nc.dram_tensor("bias", [1, F], F32, kind="ExternalInput")
    out_d = nc.dram_tensor("out", [HALF, F], F32, kind="ExternalOutput")

    with tile.TileContext(nc) as tc:
        with (
            tc.tile_pool(name="const", bufs=1) as cpool,
            tc.tile_pool(name="work", bufs=3) as wpool,
            tc.tile_pool(name="dscr", bufs=1, space="DRAM") as dpool,
            tc.tile_pool(name="psum", bufs=2, space="PSUM") as ppool,
        ):
            # ---------------- constants ----------------
            identity = cpool.tile([128, 128], F32)
            make_identity(nc, identity[:])
            ones = cpool.tile([128, 128], F32)
            nc.gpsimd.memset(ones[:], 1.0)

            # ---------------- tiny weight prep ----------------
            w_sb = cpool.tile([F, F], F32)
            nc.sync.dma_start(w_sb[:], w_d.ap())
            aa_sb = cpool.tile([F, 2], F32)
            nc.sync.dma_start(aa_sb[:], aa_d.ap())
            bias_sb = cpool.tile([1, F], F32)
            nc.sync.dma_start(bias_sb[:], bias_d.ap())

            wT_ps = ppool.tile([F, F], F32, tag="mix")
            nc.tensor.transpose(wT_ps[:], w_sb[:], identity[0:F, 0:F])
            wT_sb = cpool.tile([F, F], F32)
            nc.scalar.copy(wT_sb[:], wT_ps[:])

            # ws = w @ [a_src | a_dst]  -> [64, 2]
            ws_ps = ppool.tile([F, 2], F32, tag="mix")
            nc.tensor.matmul(ws_ps[:], wT_sb[:], aa_sb[:])
            ws_sb = cpool.tile([F, 2], F32)
            nc.scalar.copy(ws_sb[:], ws_ps[:])

            # w_aug = [w | w@a_dst | w@a_src]: h @ w_aug -> [h' | d | s]
            w_aug = cpool.tile([F, F + 2], F32)
            nc.scalar.copy(w_aug[:, 0:F], w_sb[:])
            nc.scalar.copy(w_aug[:, F : F + 1], ws_sb[:, 1:2])
            nc.scalar.copy(w_aug[:, F + 1 : F + 2], ws_sb[:, 0:1])

            # bias broadcast to all partitions
            biasb_ps = ppool.tile([128, F], F32, tag="mix")
            nc.tensor.matmul(biasb_ps[:], ones[0:1, 0:128], bias_sb[:])
            bias_rep = cpool.tile([128, F], F32)
            nc.scalar.copy(bias_rep[:], biasb_ps[:])

            # grid thresholds replicated across partitions, bf16-exact
            iota32 = cpool.tile([1, BGRID], I32)
            nc.gpsimd.iota(iota32[:], [[1, BGRID]], base=0, channel_multiplier=0)
            iota_f = cpool.tile([1, BGRID], F32)
            nc.vector.tensor_copy(iota_f[:], iota32[:])
            tgrid = cpool.tile([1, BGRID], F32)
            nc.vector.tensor_scalar(
                tgrid[:], iota_f[:], G, LO, op0=OP.mult, op1=OP.add
            )
            trow_ps = ppool.tile([128, BGRID], F32, tag="mix")
            nc.tensor.matmul(trow_ps[:], ones[0:1, 0:128], tgrid[:])
            trow_b = cpool.tile([128, BGRID], BF16)
            nc.scalar.copy(trow_b[:], trow_ps[:])

            # ---------------- h load, h^T, h' (+d +s) ----------------
            hT = cpool.tile([F, BS], F32)               # h^T (PE stationary)
            hpr = cpool.tile([128, NT_J * (F + 2)], F32)  # [h' | d | s] per tile
            hpr3 = hpr[:].rearrange("p (t c) -> p t c", c=F + 2)
            hf_view = hfull_d.ap().rearrange("(a p) f -> p a f", p=128)
            cp_engines = [nc.vector, nc.scalar, nc.gpsimd]
            for blk in range(NT_J // 8):
                ldb = wpool.tile([128, 8 * F], F32, tag="hloadb", bufs=2)
                nc.sync.dma_start(
                    ldb[:], hf_view[:, blk * 8 : (blk + 1) * 8, :]
                )
                for k in range(8):
                    jt = blk * 8 + k
                    tr = ppool.tile([F, 128], F32, tag="mix")
                    nc.tensor.transpose(
                        tr[:], ldb[:, k * F : (k + 1) * F], identity[:]
                    )
                    if k % 2 == 0:
                        nc.scalar.copy(hT[:, jt * 128 : (jt + 1) * 128], tr[:])
                    else:
                        nc.vector.tensor_copy(
                            hT[:, jt * 128 : (jt + 1) * 128], tr[:]
                        )
                    hp_ps = ppool.tile([128, F + 2], F32, tag="mix")
                    nc.tensor.matmul(
                        hp_ps[:], hT[:, jt * 128 : (jt + 1) * 128], w_aug[:]
                    )
                    if jt % 2 == 0:
                        nc.scalar.copy(hpr3[:, jt, :], hp_ps[:])
                    else:
                        nc.vector.tensor_copy(hpr3[:, jt, :], hp_ps[:])

            d_col = hpr3[:, :, F]          # [128, NT_J] strided view
            s_half = hpr3[:, 0:NT_I, F + 1]  # queries = rows 0..4095

            # ---------------- stationary tables ST = [Hv|v | Hq|q] ---------
            v_col = cpool.tile([128, NT_J], F32)
            nc.scalar.activation(v_col[:], d_col, AF.Exp)
            q_col = cpool.tile([128, NT_J], F32)
            nc.scalar.activation(q_col[:], d_col, AF.Exp, scale=ALPHA)

            ST = cpool.tile([128, NT_J * AUGW], FP16)
            ST3 = ST[:].rearrange("p (t c) -> p t c", c=AUGW)
            nc.vector.tensor_copy(ST3[:, :, F], v_col[:])
            nc.vector.tensor_copy(ST3[:, :, AUGW - 1], q_col[:])
            for jt in range(NT_J):
                te = cp_engines[jt % 3]
                if te is nc.scalar:
                    te.activation(
                        ST3[:, jt, 0:F], hpr3[:, jt, 0:F], AF.Identity,
                        scale=v_col[:, jt : jt + 1],
                    )
                    te.activation(
                        ST3[:, jt, F + 1 : AUGW - 1], hpr3[:, jt, 0:F],
                        AF.Identity, scale=q_col[:, jt : jt + 1],
                    )
                else:
                    te.tensor_scalar_mul(
                        ST3[:, jt, 0:F], hpr3[:, jt, 0:F], v_col[:, jt : jt + 1]
                    )
                    te.tensor_scalar_mul(
                        ST3[:, jt, F + 1 : AUGW - 1], hpr3[:, jt, 0:F],
                        q_col[:, jt : jt + 1],
                    )

            # ---------------- query-side prep ----------------
            # r = e^{-0.8 s}; negr = -r; bucket b = clamp((-s-LO)/G, 0, B-1)
            negr_col = cpool.tile([128, NT_I], F32)
            nc.scalar.activation(negr_col[:], s_half, AF.Exp, scale=-(1.0 - ALPHA))
            nc.vector.tensor_scalar_mul(negr_col[:], negr_col[:], -1.0)

            b_f = wpool.tile([128, NT_I], F32, tag="bf", bufs=1)
            nc.vector.tensor_scalar(
                b_f[:], s_half, -INVG, -LO * INVG, op0=OP.mult, op1=OP.add
            )
            b_cl = wpool.tile([128, NT_I], F32, tag="bcl", bufs=1)
            nc.vector.tensor_scalar(
                b_cl[:], b_f[:], 0.0, float(BGRID - 1), op0=OP.max, op1=OP.min
            )
            b_i16 = wpool.tile([128, NT_I], I16, tag="bi16", bufs=1)
            nc.vector.tensor_copy(b_i16[:], b_cl[:])

            # index layout shuffle [128,32] -> wrapped [16,256], via DRAM
            idx_dram = dpool.tile([HALF], I16, name="idxscr")
            nc.sync.dma_start(
                idx_dram[:].rearrange("(c q) -> q c", q=128), b_i16[:]
            )
            idxs = cpool.tile([80, HALF // 16], I16)
            idx_view = idx_dram[:].rearrange("(m p) -> p m", p=16)
            for g in range(5):
                nc.sync.dma_start(idxs[16 * g : 16 * (g + 1), :], idx_view)

            # ---------------- grid mask matmul ----------------
            bankV = ppool.tile([F + 1, BGRID], F32, tag="bankV", bufs=1)
            bankQ = ppool.tile([F + 1, BGRID], F32, tag="bankQ", bufs=1)
            for jt in range(NT_J):
                mw = wpool.tile([128, BGRID], FP16, tag="mask", bufs=4)
                nc.vector.tensor_scalar(
                    mw[:], trow_b[:], hpr3[:, jt, F : F + 1], None, op0=OP.is_le
                )
                st, sp = (jt == 0), (jt == NT_J - 1)
                nc.tensor.matmul(
                    bankV[:], ST3[:, jt, 0 : F + 1], mw[:], start=st, stop=sp
                )
                nc.tensor.matmul(
                    bankQ[:], ST3[:, jt, F + 1 : AUGW], mw[:], start=st, stop=sp
                )

            # tables to SBUF: A = g_v, D = g_q - Sq  (Sq = g_q[:, 0])
            sq_col = cpool.tile([F + 1, 1], F32)
            nc.vector.tensor_copy(sq_col[:], bankQ[:, 0:1])
            A_tab = cpool.tile([80, BGRID], F32)
            nc.gpsimd.memset(A_tab[:], 0.0)
            nc.scalar.copy(A_tab[0 : F + 1, :], bankV[:])
            D_tab = cpool.tile([80, BGRID], F32)
            nc.gpsimd.memset(D_tab[:], 0.0)
            nc.vector.tensor_scalar(
                D_tab[0 : F + 1, :], bankQ[:], sq_col[:, 0:1], None,
                op0=OP.subtract,
            )

            # ---------------- gather + epilogue ----------------
            NCHUNK = 4
            QC = HALF // NCHUNK          # 1024 queries per chunk
            ITC = QC // 128              # 8 i-tiles per chunk
            for ck in range(NCHUNK):
                gA = wpool.tile([80, QC], F32, tag="gA", bufs=2)
                nc.gpsimd.ap_gather(
                    gA[:].rearrange("p (n d) -> p n d", d=1),
                    A_tab[:].rearrange("p (n d) -> p n d", d=1),
                    idxs[:, ck * (QC // 16) : (ck + 1) * (QC // 16)],
                    channels=80, num_elems=BGRID, d=1, num_idxs=QC,
                )
                gD = wpool.tile([80, QC], F32, tag="gD", bufs=2)
                nc.gpsimd.ap_gather(
                    gD[:].rearrange("p (n d) -> p n d", d=1),
                    D_tab[:].rearrange("p (n d) -> p n d", d=1),
                    idxs[:, ck * (QC // 16) : (ck + 1) * (QC // 16)],
                    channels=80, num_elems=BGRID, d=1, num_idxs=QC,
                )
                for sub in range(ITC):
                    it = ck * ITC + sub
                    ac_ps = ppool.tile([128, AUGW], F32, tag="acps", bufs=2)
                    nc.tensor.transpose(
                        ac_ps[:, 0 : F + 1],
                        gA[0 : F + 1, sub * 128 : (sub + 1) * 128],
                        identity[0 : F + 1, 0 : F + 1],
                    )
                    nc.tensor.transpose(
                        ac_ps[:, F + 1 : AUGW],
                        gD[0 : F + 1, sub * 128 : (sub + 1) * 128],
                        identity[0 : F + 1, 0 : F + 1],
                    )
                    a_sb = wpool.tile([128, F + 1], F32, tag="asb", bufs=3)
                    nc.scalar.copy(a_sb[:], ac_ps[:, 0 : F + 1])
                    num = wpool.tile([128, F], F32, tag="num", bufs=3)
                    nc.vector.scalar_tensor_tensor(
                        num[:], ac_ps[:, F + 1 : AUGW - 1],
                        negr_col[:, it : it + 1], a_sb[:, 0:F],
                        op0=OP.mult, op1=OP.add,
                    )
                    den = wpool.tile([128, 1], F32, tag="den", bufs=3)
                    nc.vector.scalar_tensor_tensor(
                        den[:], ac_ps[:, AUGW - 1 : AUGW],
                        negr_col[:, it : it + 1], a_sb[:, F : F + 1],
                        op0=OP.mult, op1=OP.add,
                    )
                    rec = wpool.tile([128, 1], F32, tag="rec", bufs=3)
                    nc.vector.reciprocal(rec[:], den[:])
                    o_f = wpool.tile([128, F], F32, tag="of", bufs=3)
                    nc.vector.scalar_tensor_tensor(
                        o_f[:], num[:], rec[:, 0:1], bias_rep[:],
                        op0=OP.mult, op1=OP.add,
                    )
                    nc.sync.dma_start(
                        out_d.ap()[it * 128 : (it + 1) * 128, :], o_f[:]
                    )

    nc.compile()
    return nc


_NC_CACHE = None


def _get_nc():
    global _NC_CACHE
    if _NC_CACHE is None:
        _NC_CACHE = _build_kernel_module()
    return _NC_CACHE


def _make_in_maps(h, w, a_src, a_dst, bias):
    h = np.ascontiguousarray(np.asarray(h, dtype=np.float32))
    w = np.asarray(w, dtype=np.float32)
    a_src = np.asarray(a_src, dtype=np.float32)
    a_dst = np.asarray(a_dst, dtype=np.float32)
    bias = np.asarray(bias, dtype=np.float32).reshape(1, F)
    in_maps = []
    for c in range(8):
        head, half = c // 2, c % 2
        aa = np.ascontiguousarray(
            np.concatenate([a_src[head], a_dst[head]], axis=1)
        )
        # rotate rows so this core's query half is rows 0..HALF-1
        hrot = np.ascontiguousarray(
            np.concatenate([h[half * HALF :], h[: half * HALF]])
        )
        in_maps.append(
            {
                "hfull": hrot,
                "w": np.ascontiguousarray(w[head]),
                "aa": aa,
                "bias": bias,
            }
        )
    return in_maps


def _run(h, w, a_src, a_dst, bias, trace=False, **trace_kwargs):
    nc = _get_nc()
    in_maps = _make_in_maps(h, w, a_src, a_dst, bias)
    res = run_bass_kernel_spmd(
        nc, in_maps, core_ids=list(range(8)), trace=trace, **trace_kwargs
    )
    out = np.zeros((BS, NH * F), dtype=np.float32)
    for c in range(8):
        head, half = c // 2, c % 2
        out[half * HALF : (half + 1) * HALF, head * F : (head + 1) * F] = res.results[
            c
        ]["out"]
    return out, res


def kernel(h, w, a_src, a_dst, bias):
    out, _ = _run(h, w, a_src, a_dst, bias, trace=False)
    return out


# revision 19
# speedup vs baseline: 2.3623x; 2.3623x over previous
"""Multi-head graph-attention (GAT) kernel for Trainium2, 8 NeuronCores.

Reference computation (per head):
    h_prime = h @ w[head]                       # [8192, 64]
    s = h_prime @ a_src[head],  d = h_prime @ a_dst[head]
    attn = softmax_j(leaky_relu(s_i + d_j, 0.2))
    out  = attn @ h_prime + bias                # -> [8192, 4*64]

Key identity: with exp monotone,
    exp(lrelu(s_i + d_j)) = e^{s_i} e^{d_j}           if s_i + d_j >= 0
                          = e^{0.2 s_i} e^{0.2 d_j}   otherwise
The mask sets {j : d_j >= -s_i} are NESTED across i, so the whole O(n^2)
attention contraction is a 1-D step function of t = -s_i:
    g(t) = sum_{j: d_j >= t} [v_j h'_j | v_j],   v = e^d   (and gq with q)
Evaluate g on a fixed grid of B=128 thresholds t_b (one fp16 matmul with
only B moving columns per 128-row j-tile), then each query i picks its
column b_i = clamp((-s_i - LO)/G) via a one-hot matmul that lands the
result directly in query-partition layout.  Grid rounding only
misclassifies js with |s_i + d_j| <= G, where exp(lrelu(x)) is
continuous, so the error is O(G^2) ~ 3e-4.  Everything else is exact:
    num_i / e^{s_i} = A(b_i) - r_i (C(b_i) - Sq),   r_i = e^{-0.8 s_i}
    den_i / e^{s_i} = a(b_i) - r_i (c(b_i) - sq)
with A|a = g_v columns, C|c = g_q columns, Sq|sq the full q-sums
(= g_q at the leftmost grid point).

Sharding: 8 cores = 4 heads x 2 query-halves. Each core gets the full h
(rows rotated so its query half is rows 0..4095 - j-side sums are order
invariant), computes the full j-side tables, and evaluates its 4096
queries. No collectives.
"""

import numpy as np

import concourse.bass as bass
import concourse.tile as tile
from concourse import bacc, mybir
from concourse.bass_utils import run_bass_kernel_spmd
from concourse.masks import make_identity

F32 = mybir.dt.float32
F32R = mybir.dt.float32r
BF16 = mybir.dt.bfloat16
FP16 = mybir.dt.float16
I32 = mybir.dt.int32
AF = mybir.ActivationFunctionType
OP = mybir.AluOpType

BS = 8192          # nodes
F = 64             # f_in == f_out
NH = 4             # heads
HALF = BS // 2     # queries per core
NT_J = BS // 128   # 64 j tiles
NT_I = HALF // 128 # 32 query tiles
ALPHA = 0.2

BGRID = 128        # grid points (one PE tile)
LO = -8.0          # grid range [LO, -LO)
G = (-2.0 * LO) / BGRID   # 1/8
INVG = 1.0 / G

AUGW = 2 * (F + 1)  # 130: [Hv|v | Hq|q] stationary width


def _build_kernel_module():
    nc = bacc.Bacc("TRN2", target_bir_lowering=False, debug=False)

    hfull_d = nc.dram_tensor("hfull", [BS, F], F32, kind="ExternalInput")
    w_d = nc.dram_tensor("w", [F, F], F32, kind="ExternalInput")
    aa_d = nc.dram_tensor("aa", [F, 2], F32, kind="ExternalInput")
    bias_d = nc.dram_tensor("bias", [1, F], F32, kind="ExternalInput")
    out_d = nc.dram_tensor("out", [HALF, F], F32, kind="ExternalOutput")

    with tile.TileContext(nc) as tc:
        with (
            tc.tile_pool(name="const", bufs=1) as cpool,
            tc.tile_pool(name="work", bufs=3) as wpool,
            tc.tile_pool(name="dscr", bufs=1, space="DRAM") as dpool,
            tc.tile_pool(name="psum", bufs=2, space="PSUM") as ppool,
        ):
            # ---------------- constants ----------------
            identity = cpool.tile([128, 128], F32)
            make_identity(nc, identity[:])
            ones = cpool.tile([128, 128], F32)
            nc.gpsimd.memset(ones[:], 1.0)
            iota_col = cpool.tile([128, 1], I32)
            nc.gpsimd.iota(iota_col[:], [[1, 1]], base=0, channel_multiplier=1)
            iota_f = cpool.tile([128, 1], F32)
            nc.vector.tensor_copy(iota_f[:], iota_col[:])

            # ---------------- tiny weight prep ----------------
            w_sb = cpool.tile([F, F], F32)
            nc.sync.dma_start(w_sb[:], w_d.ap())
            aa_sb = cpool.tile([F, 2], F32)
            nc.sync.dma_start(aa_sb[:], aa_d.ap())
            bias_sb = cpool.tile([1, F], F32)
            nc.sync.dma_start(bias_sb[:], bias_d.ap())

            wT_ps = ppool.tile([F, F], F32, tag="mix")
            nc.tensor.transpose(wT_ps[:], w_sb[:], identity[0:F, 0:F])
            wT_sb = cpool.tile([F, F], F32)
            nc.scalar.copy(wT_sb[:], wT_ps[:])

            # ws = w @ [a_src | a_dst]  -> [64, 2]
            ws_ps = ppool.tile([F, 2], F32, tag="mix")
            nc.tensor.matmul(ws_ps[:], wT_sb[:], aa_sb[:])
            ws_sb = cpool.tile([F, 2], F32)
            nc.scalar.copy(ws_sb[:], ws_ps[:])

            # w_aug = [w | w@a_dst | w@a_src]: h @ w_aug -> [h' | d | s]
            # replicated on partitions 64:128 for odd-tile stationaries
            w_top = cpool.tile([F, F + 2], F32R)
            nc.scalar.copy(w_top[:, 0:F], w_sb[:])
            nc.scalar.copy(w_top[:, F : F + 1], ws_sb[:, 1:2])
            nc.scalar.copy(w_top[:, F + 1 : F + 2], ws_sb[:, 0:1])
            dbl_id = cpool.tile([F, 128], F32R)
            nc.scalar.copy(dbl_id[:, 0:F], identity[0:F, 0:F])
            nc.vector.tensor_copy(dbl_id[:, F:128], identity[0:F, 0:F])
            wa_ps = ppool.tile([128, F + 2], F32, tag="mix")
            nc.tensor.matmul(wa_ps[:], dbl_id[:], w_top[:])
            w_aug = cpool.tile([128, F + 2], F32R)
            nc.scalar.copy(w_aug[:], wa_ps[:])

            # bias broadcast to all partitions
            biasb_ps = ppool.tile([128, F], F32, tag="mix")
            nc.tensor.matmul(biasb_ps[:], ones[0:1, 0:128], bias_sb[:])
            bias_rep = cpool.tile([128, F], F32)
            nc.scalar.copy(bias_rep[:], biasb_ps[:])

            # grid thresholds replicated across partitions, bf16-exact
            tg_f = cpool.tile([1, BGRID], F32)
            nc.gpsimd.iota(
                tg_f[:], [[1, BGRID]], base=0, channel_multiplier=0,
                allow_small_or_imprecise_dtypes=True,
            )
            tgrid = cpool.tile([1, BGRID], F32)
            nc.vector.tensor_scalar(
                tgrid[:], tg_f[:], G, LO, op0=OP.mult, op1=OP.add
            )
            trow_ps = ppool.tile([128, BGRID], F32, tag="mix")
            nc.tensor.matmul(trow_ps[:], ones[0:1, 0:128], tgrid[:])
            trow_b = cpool.tile([128, BGRID], BF16)
            nc.scalar.copy(trow_b[:], trow_ps[:])

            # ---------------- h load, h^T (paired), h' (+d +s) -------------
            # hT2 chunk p holds transposed tiles 2p (partitions 0:64) and
            # 2p+1 (partitions 64:128).
            hT2 = cpool.tile([128, (NT_J // 2) * 128], F32R)
            hprB = cpool.tile([128, NT_J * F], BF16)   # h' in bf16
            hprB3 = hprB[:].rearrange("p (t c) -> p t c", c=F)
            ds_col = cpool.tile([128, NT_J * 2], F32)  # [d | s] per tile
            ds3 = ds_col[:].rearrange("p (t c) -> p t c", c=2)
            hf_view = hfull_d.ap().rearrange("(a p) f -> p a f", p=128)
            for blk in range(NT_J // 8):
                ldb = wpool.tile([128, 8 * F], F32, tag="hloadb", bufs=2)
                nc.sync.dma_start(
                    ldb[:], hf_view[:, blk * 8 : (blk + 1) * 8, :]
                )
                for k in range(4):
                    pr = blk * 4 + k  # pair index
                    tr = ppool.tile([128, 128], F32, tag="mix")
                    nc.tensor.transpose(
                        tr[:],
                        ldb[:, k * 128 : (k + 1) * 128],
                        identity[:],
                    )
                    if k % 2 == 0:
                        nc.scalar.copy(hT2[:, pr * 128 : (pr + 1) * 128], tr[:])
                    else:
                        nc.vector.tensor_copy(
                            hT2[:, pr * 128 : (pr + 1) * 128], tr[:]
                        )
                for k in range(8):
                    jt = blk * 8 + k
                    pr, par = jt // 2, (jt % 2) * F
                    hp_ps = ppool.tile([128, F + 2], F32, tag="mix")
                    nc.tensor.matmul(
                        hp_ps[:],
                        hT2[par : par + F, pr * 128 : (pr + 1) * 128],
                        w_aug[par : par + F, :],
                    )
                    if k % 2 == 0:
                        nc.scalar.copy(hprB3[:, jt, :], hp_ps[:, 0:F])
                        nc.vector.tensor_copy(ds3[:, jt, :], hp_ps[:, F : F + 2])
                    else:
                        nc.vector.tensor_copy(hprB3[:, jt, :], hp_ps[:, 0:F])
                        nc.scalar.copy(ds3[:, jt, :], hp_ps[:, F : F + 2])

            d_all = ds3[:, :, 0]            # [128, NT_J] strided
            s_half = ds3[:, 0:NT_I, 1]      # queries = rows 0..4095

            # ---------------- stationary tables ST = [Hv|v | Hq|q] ---------
            v_col = cpool.tile([128, NT_J], F32)
            nc.scalar.activation(v_col[:], d_all, AF.Exp)
            q_col = cpool.tile([128, NT_J], F32)
            nc.scalar.activation(q_col[:], d_all, AF.Exp, scale=ALPHA)

            ST = cpool.tile([128, NT_J * AUGW], FP16)
            ST3 = ST[:].rearrange("p (t c) -> p t c", c=AUGW)
            nc.vector.tensor_copy(ST3[:, :, F], v_col[:])
            nc.vector.tensor_copy(ST3[:, :, AUGW - 1], q_col[:])
            for jt in range(NT_J):
                if jt % 3 == 2:
                    nc.scalar.activation(
                        ST3[:, jt, 0:F], hprB3[:, jt, :], AF.Identity,
                        scale=v_col[:, jt : jt + 1],
                    )
                    nc.scalar.activation(
                        ST3[:, jt, F + 1 : AUGW - 1], hprB3[:, jt, :],
                        AF.Identity, scale=q_col[:, jt : jt + 1],
                    )
                else:
                    te = nc.vector if jt % 3 == 0 else nc.gpsimd
                    te.tensor_scalar_mul(
                        ST3[:, jt, 0:F], hprB3[:, jt, :], v_col[:, jt : jt + 1]
                    )
                    te.tensor_scalar_mul(
                        ST3[:, jt, F + 1 : AUGW - 1], hprB3[:, jt, :],
                        q_col[:, jt : jt + 1],
                    )

            # ---------------- query-side prep ----------------
            negr_col = cpool.tile([128, NT_I], F32)
            nc.scalar.activation(negr_col[:], s_half, AF.Exp, scale=-(1.0 - ALPHA))
            nc.vector.tensor_scalar_mul(negr_col[:], negr_col[:], -1.0)

            b_f = wpool.tile([128, NT_I], F32, tag="bf", bufs=1)
            nc.vector.tensor_scalar(
                b_f[:], s_half, -INVG, -LO * INVG, op0=OP.mult, op1=OP.add
            )
            b_rnd = wpool.tile([128, NT_I], F32, tag="brnd", bufs=1)
            nc.vector.tensor_scalar(
                b_rnd[:], b_f[:], 8388608.0, 8388608.0,
                op0=OP.add, op1=OP.subtract,
            )
            b_cl = wpool.tile([128, NT_I], BF16, tag="bcl", bufs=1)
            nc.vector.tensor_scalar(
                b_cl[:], b_rnd[:], 0.0, float(BGRID - 1), op0=OP.max, op1=OP.min
            )
            # roundtrip through DRAM to get b as a single row [1, 4096]
            b_dram = dpool.tile([HALF], BF16, name="bscr")
            nc.sync.dma_start(
                b_dram[:].rearrange("(c q) -> q c", q=128), b_cl[:]
            )
            b_row = cpool.tile([1, HALF], BF16)
            ones_bf = cpool.tile([1, 128], BF16)
            nc.vector.tensor_copy(ones_bf[:], ones[0:1, 0:128])
            nc.sync.dma_start(b_row[:], b_dram[:].rearrange("(a n) -> a n", a=1))

            # ---------------- grid mask matmul ----------------
            bankV = ppool.tile([F + 1, BGRID], F32, tag="bankV", bufs=1)
            bankQ = ppool.tile([F + 1, BGRID], F32, tag="bankQ", bufs=1)
            for jt in range(NT_J):
                mw = wpool.tile([128, BGRID], FP16, tag="mask", bufs=4)
                nc.vector.tensor_scalar(
                    mw[:], trow_b[:], ds3[:, jt, 0:1], None, op0=OP.is_le
                )
                st, sp = (jt == 0), (jt == NT_J - 1)
                nc.tensor.matmul(
                    bankV[:], ST3[:, jt, 0 : F + 1], mw[:], start=st, stop=sp
                )
                nc.tensor.matmul(
                    bankQ[:], ST3[:, jt, F + 1 : AUGW], mw[:], start=st, stop=sp
                )

            # tables: A = g_v, D = g_q - Sq  (Sq = g_q[:, 0]); transpose to
            # [grid-part, comp] fp16 for the one-hot matmuls
            sq_col = cpool.tile([F + 1, 1], F32)
            nc.vector.tensor_copy(sq_col[:], bankQ[:, 0:1])
            A_sb = cpool.tile([F + 1, BGRID], F32)
            nc.scalar.copy(A_sb[:], bankV[:])
            D_sb = cpool.tile([F + 1, BGRID], F32)
            nc.vector.tensor_scalar(
                D_sb[:], bankQ[:], sq_col[:, 0:1], None, op0=OP.subtract
            )
            gvT_ps = ppool.tile([BGRID, F + 1], F32, tag="mix")
            nc.tensor.transpose(
                gvT_ps[:], A_sb[:],
                identity[0 : F + 1, 0 : F + 1],
            )
            gvT = cpool.tile([BGRID, F + 1], FP16)
            nc.scalar.copy(gvT[:], gvT_ps[:])
            gdT_ps = ppool.tile([BGRID, F + 1], F32, tag="mix")
            nc.tensor.transpose(
                gdT_ps[:], D_sb[:],
                identity[0 : F + 1, 0 : F + 1],
            )
            gdT = cpool.tile([BGRID, F + 1], FP16)
            nc.vector.tensor_copy(gdT[:], gdT_ps[:])

            # one-hot of query buckets: oh[b, i] = (b_i == b), fp16
            b_rep = cpool.tile([128, HALF], BF16)
            for ch in range(8):
                br_ps = ppool.tile([128, 512], F32, tag="mix")
                nc.tensor.matmul(
                    br_ps[:],
                    ones_bf[:],
                    b_row[:, ch * 512 : (ch + 1) * 512],
                )
                if ch % 2 == 0:
                    nc.scalar.copy(b_rep[:, ch * 512 : (ch + 1) * 512], br_ps[:])
                else:
                    nc.vector.tensor_copy(
                        b_rep[:, ch * 512 : (ch + 1) * 512], br_ps[:]
                    )
            oh = cpool.tile([128, HALF], FP16)
            nc.vector.tensor_scalar(
                oh[:], b_rep[:], iota_f[:, 0:1], None, op0=OP.is_equal
            )

            # ---------------- per-tile one-hot gather + epilogue -----------
            o_all = cpool.tile([128, NT_I * F], F32)
            o3 = o_all[:].rearrange("p (t c) -> p t c", c=F)
            out_view = out_d.ap().rearrange("(a p) f -> p a f", p=128)
            for it in range(NT_I):
                a_ps = ppool.tile([128, F + 1], F32, tag="aps", bufs=2)
                d_ps = ppool.tile([128, F + 1], F32, tag="dps", bufs=2)
                nc.tensor.matmul(
                    a_ps[:], oh[:, it * 128 : (it + 1) * 128], gvT[:]
                )
                i2 = nc.tensor.matmul(
                    d_ps[:], oh[:, it * 128 : (it + 1) * 128], gdT[:]
                )
                i2.ins.ldweights = False
                a_sb = wpool.tile([128, F + 1], F32, tag="asb", bufs=3)
                nc.scalar.copy(a_sb[:], a_ps[:])
                num = wpool.tile([128, F], F32, tag="num", bufs=3)
                nc.vector.scalar_tensor_tensor(
                    num[:], d_ps[:, 0:F], negr_col[:, it : it + 1],
                    a_sb[:, 0:F], op0=OP.mult, op1=OP.add,
                )
                den = wpool.tile([128, 1], F32, tag="den", bufs=3)
                nc.vector.scalar_tensor_tensor(
                    den[:], d_ps[:, F : F + 1], negr_col[:, it : it + 1],
                    a_sb[:, F : F + 1], op0=OP.mult, op1=OP.add,
                )
                rec = wpool.tile([128, 1], F32, tag="rec", bufs=3)
                nc.vector.reciprocal(rec[:], den[:])
                o_t = wpool.tile([128, F], F32, tag="ot", bufs=3)
                nc.gpsimd.tensor_scalar_mul(o_t[:], num[:], rec[:, 0:1])
                nc.gpsimd.tensor_add(o3[:, it, :], o_t[:], bias_rep[:])
                if it % 8 == 7:
                    grp = it // 8
                    nc.sync.dma_start(
                        out_view[:, grp * 8 : (grp + 1) * 8, :],
                        o_all[:, grp * 8 * F : (grp + 1) * 8 * F],
                    )

    nc.compile()
    return nc


_NC_CACHE = None


def _get_nc():
    global _NC_CACHE
    if _NC_CACHE is None:
        _NC_CACHE = _build_kernel_module()
    return _NC_CACHE


def _make_in_maps(h, w, a_src, a_dst, bias):
    h = np.ascontiguousarray(np.asarray(h, dtype=np.float32))
    w = np.asarray(w, dtype=np.float32)
    a_src = np.asarray(a_src, dtype=np.float32)
    a_dst = np.asarray(a_dst, dtype=np.float32)
    bias = np.asarray(bias, dtype=np.float32).reshape(1, F)
    in_maps = []
    for c in range(8):
        head, half = c // 2, c % 2
        aa = np.ascontiguousarray(
            np.concatenate([a_src[head], a_dst[head]], axis=1)
        )
        # rotate rows so this core's query half is rows 0..HALF-1
        hrot = np.ascontiguousarray(
            np.concatenate([h[half * HALF :], h[: half * HALF]])
        )
        in_maps.append(
            {
                "hfull": hrot,
                "w": np.ascontiguousarray(w[head]),
                "aa": aa,
                "bias": bias,
            }
        )
    return in_maps


def _run(h, w, a_src, a_dst, bias, trace=False, **trace_kwargs):
    nc = _get_nc()
    in_maps = _make_in_maps(h, w, a_src, a_dst, bias)
    res = run_bass_kernel_spmd(
        nc, in_maps, core_ids=list(range(8)), trace=trace, **trace_kwargs
    )
    out = np.zeros((BS, NH * F), dtype=np.float32)
    for c in range(8):
        head, half = c // 2, c % 2
        out[half * HALF : (half + 1) * HALF, head * F : (head + 1) * F] = res.results[
            c
        ]["out"]
    return out, res


def kernel(h, w, a_src, a_dst, bias):
    out, _ = _run(h, w, a_src, a_dst, bias, trace=False)
    return out


# revision 20
# speedup vs baseline: 3.5981x; 1.5231x over previous
"""Multi-head graph-attention (GAT) kernel for Trainium2, 8 NeuronCores.

Reference computation (per head):
    h_prime = h @ w[head]                       # [8192, 64]
    s = h_prime @ a_src[head],  d = h_prime @ a_dst[head]
    attn = softmax_j(leaky_relu(s_i + d_j, 0.2))
    out  = attn @ h_prime + bias                # -> [8192, 4*64]

Key identity: with exp monotone,
    exp(lrelu(s_i + d_j)) = e^{s_i} e^{d_j}           if s_i + d_j >= 0
                          = e^{0.2 s_i} e^{0.2 d_j}   otherwise
The mask sets {j : d_j >= -s_i} are NESTED across i, so the whole O(n^2)
attention contraction is a 1-D step function of t = -s_i:
    g(t) = sum_{j: d_j >= t} [v_j h'_j | v_j],   v = e^d   (and gq with q)
Evaluate g on a fixed grid of B=128 thresholds t_b (one fp16 matmul with
only B moving columns per 128-row j-tile), then each query i picks its
column b_i = clamp((-s_i - LO)/G) via a one-hot matmul that lands the
result directly in query-partition layout.  Grid rounding only
misclassifies js with |s_i + d_j| <= G, where exp(lrelu(x)) is
continuous, so the error is O(G^2) ~ 3e-4.  Everything else is exact:
    num_i / e^{s_i} = A(b_i) - r_i (C(b_i) - Sq),   r_i = e^{-0.8 s_i}
    den_i / e^{s_i} = a(b_i) - r_i (c(b_i) - sq)
with A|a = g_v columns, C|c = g_q columns, Sq|sq the full q-sums
(= g_q at the leftmost grid point).

Sharding: 8 cores = 4 heads x 2 query-halves. Each core gets the full h
(rows rotated so its query half is rows 0..4095 - j-side sums are order
invariant), computes the full j-side tables, and evaluates its 4096
queries. No collectives.
"""

import numpy as np

import concourse.bass as bass
import concourse.tile as tile
from concourse import bacc, mybir
from concourse.bass_utils import run_bass_kernel_spmd
from concourse.masks import make_identity

F32 = mybir.dt.float32
F32R = mybir.dt.float32r
BF16 = mybir.dt.bfloat16
FP16 = mybir.dt.float16
I32 = mybir.dt.int32
AF = mybir.ActivationFunctionType
OP = mybir.AluOpType

BS = 8192          # nodes
F = 64             # f_in == f_out
NH = 4             # heads
HALF = BS // 2     # queries per core
NT_J = BS // 128   # 64 j tiles
NT_I = HALF // 128 # 32 query tiles
ALPHA = 0.2

BGRID = 128        # grid points (one PE tile)
LO = -8.0          # grid range [LO, -LO)
G = (-2.0 * LO) / BGRID   # 1/8
INVG = 1.0 / G

AUGW = 2 * (F + 1)  # 130: [Hv|v | Hq|q] stationary width


def _build_kernel_module():
    nc = bacc.Bacc("TRN2", target_bir_lowering=False, debug=False)

    hfull_d = nc.dram_tensor("hfull", [BS, F], F32, kind="ExternalInput")
    w_d = nc.dram_tensor("w", [F, F], F32, kind="ExternalInput")
    aa_d = nc.dram_tensor("aa", [F, 2], F32, kind="ExternalInput")
    bias_d = nc.dram_tensor("bias", [1, F], F32, kind="ExternalInput")
    out_d = nc.dram_tensor("out", [HALF, F], F32, kind="ExternalOutput")

    with tile.TileContext(nc) as tc:
        with (
            tc.tile_pool(name="const", bufs=1) as cpool,
            tc.tile_pool(name="work", bufs=3) as wpool,
            tc.tile_pool(name="dscr", bufs=1, space="DRAM") as dpool,
            tc.tile_pool(name="psum", bufs=2, space="PSUM") as ppool,
        ):
            # ---------------- constants ----------------
            identity = cpool.tile([128, 128], F32)
            make_identity(nc, identity[:])
            ones = cpool.tile([128, 128], F32)
            nc.gpsimd.memset(ones[:], 1.0)
            iota_col = cpool.tile([128, 1], I32)
            nc.gpsimd.iota(iota_col[:], [[1, 1]], base=0, channel_multiplier=1)
            iota_f = cpool.tile([128, 1], F32)
            nc.vector.tensor_copy(iota_f[:], iota_col[:])

            # ---------------- tiny weight prep ----------------
            w_sb = cpool.tile([F, F], F32)
            nc.sync.dma_start(w_sb[:], w_d.ap())
            aa_sb = cpool.tile([F, 2], F32)
            nc.sync.dma_start(aa_sb[:], aa_d.ap())
            bias_sb = cpool.tile([1, F], F32)
            nc.sync.dma_start(bias_sb[:], bias_d.ap())

            wT_ps = ppool.tile([F, F], F32, tag="mix")
            nc.tensor.transpose(wT_ps[:], w_sb[:], identity[0:F, 0:F])
            wT_sb = cpool.tile([F, F], F32)
            nc.scalar.copy(wT_sb[:], wT_ps[:])

            # ws = w @ [a_src | a_dst]  -> [64, 2]
            ws_ps = ppool.tile([F, 2], F32, tag="mix")
            nc.tensor.matmul(ws_ps[:], wT_sb[:], aa_sb[:])
            ws_sb = cpool.tile([F, 2], F32)
            nc.scalar.copy(ws_sb[:], ws_ps[:])

            # w_aug = [w | w@a_dst | w@a_src]: h @ w_aug -> [h' | d | s]
            # replicated on partitions 64:128 for odd-tile stationaries
            w_top = cpool.tile([F, F + 2], F32R)
            nc.scalar.copy(w_top[:, 0:F], w_sb[:])
            nc.scalar.copy(w_top[:, F : F + 1], ws_sb[:, 1:2])
            nc.scalar.copy(w_top[:, F + 1 : F + 2], ws_sb[:, 0:1])
            dbl_id = cpool.tile([F, 128], F32R)
            nc.scalar.copy(dbl_id[:, 0:F], identity[0:F, 0:F])
            nc.vector.tensor_copy(dbl_id[:, F:128], identity[0:F, 0:F])
            wa_ps = ppool.tile([128, F + 2], F32, tag="mix")
            nc.tensor.matmul(wa_ps[:], dbl_id[:], w_top[:])
            w_aug = cpool.tile([128, F + 2], F32R)
            nc.scalar.copy(w_aug[:], wa_ps[:])

            # bias broadcast to all partitions
            biasb_ps = ppool.tile([128, F], F32, tag="mix")
            nc.tensor.matmul(biasb_ps[:], ones[0:1, 0:128], bias_sb[:])
            bias_rep = cpool.tile([128, F], F32)
            nc.scalar.copy(bias_rep[:], biasb_ps[:])

            # grid thresholds replicated across partitions, bf16-exact
            tg_f = cpool.tile([1, BGRID], F32)
            nc.gpsimd.iota(
                tg_f[:], [[1, BGRID]], base=0, channel_multiplier=0,
                allow_small_or_imprecise_dtypes=True,
            )
            tgrid = cpool.tile([1, BGRID], F32)
            nc.vector.tensor_scalar(
                tgrid[:], tg_f[:], G, LO, op0=OP.mult, op1=OP.add
            )
            trow_ps = ppool.tile([128, BGRID], F32, tag="mix")
            nc.tensor.matmul(trow_ps[:], ones[0:1, 0:128], tgrid[:])
            trow_b = cpool.tile([128, BGRID], BF16)
            nc.scalar.copy(trow_b[:], trow_ps[:])

            # ---------------- h load, h^T (paired), h' (+d +s) -------------
            # hT2 chunk p holds transposed tiles 2p (partitions 0:64) and
            # 2p+1 (partitions 64:128).
            hT2 = cpool.tile([128, (NT_J // 2) * 128], F32R)
            hprB = cpool.tile([128, NT_J * F], BF16)   # h' in bf16
            hprB3 = hprB[:].rearrange("p (t c) -> p t c", c=F)
            ds_col = cpool.tile([128, NT_J * 2], F32)  # [d | s] per tile
            ds3 = ds_col[:].rearrange("p (t c) -> p t c", c=2)
            hf_view = hfull_d.ap().rearrange("(a p) f -> p a f", p=128)
            for blk in range(NT_J // 8):
                ldb = wpool.tile([128, 8 * F], F32, tag="hloadb", bufs=3)
                nc.sync.dma_start(
                    ldb[:], hf_view[:, blk * 8 : (blk + 1) * 8, :]
                )
                for k in range(4):
                    pr = blk * 4 + k  # pair index
                    tr = ppool.tile([128, 128], F32, tag="mix")
                    nc.tensor.transpose(
                        tr[:],
                        ldb[:, k * 128 : (k + 1) * 128],
                        identity[:],
                    )
                    if k % 2 == 0:
                        nc.scalar.copy(hT2[:, pr * 128 : (pr + 1) * 128], tr[:])
                    else:
                        nc.vector.tensor_copy(
                            hT2[:, pr * 128 : (pr + 1) * 128], tr[:]
                        )
                for k in range(8):
                    jt = blk * 8 + k
                    pr, par = jt // 2, (jt % 2) * F
                    hp_ps = ppool.tile([128, F + 2], F32, tag="mix")
                    nc.tensor.matmul(
                        hp_ps[:],
                        hT2[par : par + F, pr * 128 : (pr + 1) * 128],
                        w_aug[par : par + F, :],
                    )
                    if k % 2 == 0:
                        nc.scalar.copy(hprB3[:, jt, :], hp_ps[:, 0:F])
                        nc.vector.tensor_copy(ds3[:, jt, :], hp_ps[:, F : F + 2])
                    else:
                        nc.vector.tensor_copy(hprB3[:, jt, :], hp_ps[:, 0:F])
                        nc.scalar.copy(ds3[:, jt, :], hp_ps[:, F : F + 2])

            d_all = ds3[:, :, 0]            # [128, NT_J] strided
            s_half = ds3[:, 0:NT_I, 1]      # queries = rows 0..4095

            # ---------------- stationary tables ST = [Hv|v | Hq|q] ---------
            v_col = cpool.tile([128, NT_J], F32)
            nc.scalar.activation(v_col[:], d_all, AF.Exp)
            q_col = cpool.tile([128, NT_J], F32)
            nc.scalar.activation(q_col[:], d_all, AF.Exp, scale=ALPHA)

            ST = cpool.tile([128, NT_J * AUGW], FP16)
            ST3 = ST[:].rearrange("p (t c) -> p t c", c=AUGW)
            nc.vector.tensor_copy(ST3[:, :, F], v_col[:])
            nc.vector.tensor_copy(ST3[:, :, AUGW - 1], q_col[:])
            for jt in range(NT_J):
                if jt % 4 == 3:
                    nc.scalar.activation(
                        ST3[:, jt, 0:F], hprB3[:, jt, :], AF.Identity,
                        scale=v_col[:, jt : jt + 1],
                    )
                    nc.scalar.activation(
                        ST3[:, jt, F + 1 : AUGW - 1], hprB3[:, jt, :],
                        AF.Identity, scale=q_col[:, jt : jt + 1],
                    )
                else:
                    nc.vector.tensor_scalar_mul(
                        ST3[:, jt, 0:F], hprB3[:, jt, :], v_col[:, jt : jt + 1]
                    )
                    nc.vector.tensor_scalar_mul(
                        ST3[:, jt, F + 1 : AUGW - 1], hprB3[:, jt, :],
                        q_col[:, jt : jt + 1],
                    )

            # ---------------- query-side prep ----------------
            negr_col = cpool.tile([128, NT_I], F32)
            nc.scalar.activation(negr_col[:], s_half, AF.Exp, scale=-(1.0 - ALPHA))
            nc.vector.tensor_scalar_mul(negr_col[:], negr_col[:], -1.0)

            b_f = wpool.tile([128, NT_I], F32, tag="bf", bufs=1)
            nc.vector.tensor_scalar(
                b_f[:], s_half, -INVG, -LO * INVG, op0=OP.mult, op1=OP.add
            )
            b_rnd = wpool.tile([128, NT_I], F32, tag="brnd", bufs=1)
            nc.vector.tensor_scalar(
                b_rnd[:], b_f[:], 8388608.0, 8388608.0,
                op0=OP.add, op1=OP.subtract,
            )
            b_cl = wpool.tile([128, NT_I], BF16, tag="bcl", bufs=1)
            nc.vector.tensor_scalar(
                b_cl[:], b_rnd[:], 0.0, float(BGRID - 1), op0=OP.max, op1=OP.min
            )
            # roundtrip through DRAM to get b as a single row [1, 4096]
            b_dram = dpool.tile([HALF], BF16, name="bscr")
            nc.sync.dma_start(
                b_dram[:].rearrange("(c q) -> q c", q=128), b_cl[:]
            )
            b_row = cpool.tile([1, HALF], BF16)
            ones_bf = cpool.tile([1, 128], BF16)
            nc.vector.tensor_copy(ones_bf[:], ones[0:1, 0:128])
            nc.sync.dma_start(b_row[:], b_dram[:].rearrange("(a n) -> a n", a=1))

            # ---------------- grid mask matmul ----------------
            bankV = ppool.tile([F + 1, BGRID], F32, tag="bankV", bufs=1)
            bankQ = ppool.tile([F + 1, BGRID], F32, tag="bankQ", bufs=1)
            for jt in range(NT_J):
                mw = wpool.tile([128, BGRID], FP16, tag="mask", bufs=4)
                nc.vector.tensor_scalar(
                    mw[:], trow_b[:], ds3[:, jt, 0:1], None, op0=OP.is_le
                )
                st, sp = (jt == 0), (jt == NT_J - 1)
                nc.tensor.matmul(
                    bankV[:], ST3[:, jt, 0 : F + 1], mw[:], start=st, stop=sp
                )
                nc.tensor.matmul(
                    bankQ[:], ST3[:, jt, F + 1 : AUGW], mw[:], start=st, stop=sp
                )

            # tables: A = g_v, D = g_q - Sq  (Sq = g_q[:, 0]); transpose to
            # [grid-part, comp] fp16 for the one-hot matmuls
            sq_col = cpool.tile([F + 1, 1], F32)
            nc.vector.tensor_copy(sq_col[:], bankQ[:, 0:1])
            A_sb = cpool.tile([F + 1, BGRID], F32)
            nc.scalar.copy(A_sb[:], bankV[:])
            D_sb = cpool.tile([F + 1, BGRID], F32)
            nc.vector.tensor_scalar(
                D_sb[:], bankQ[:], sq_col[:, 0:1], None, op0=OP.subtract
            )
            gvT_ps = ppool.tile([BGRID, F + 1], F32, tag="mix")
            nc.tensor.transpose(
                gvT_ps[:], A_sb[:],
                identity[0 : F + 1, 0 : F + 1],
            )
            gvT = cpool.tile([BGRID, F + 1], FP16)
            nc.scalar.copy(gvT[:], gvT_ps[:])
            gdT_ps = ppool.tile([BGRID, F + 1], F32, tag="mix")
            nc.tensor.transpose(
                gdT_ps[:], D_sb[:],
                identity[0 : F + 1, 0 : F + 1],
            )
            gdT = cpool.tile([BGRID, F + 1], FP16)
            nc.vector.tensor_copy(gdT[:], gdT_ps[:])

            # one-hot of query buckets: oh[b, i] = (b_i == b), fp16
            oh = cpool.tile([128, HALF], FP16)
            for ch in range(8):
                br_ps = ppool.tile([128, 512], F32, tag="mix")
                nc.tensor.matmul(
                    br_ps[:],
                    ones_bf[:],
                    b_row[:, ch * 512 : (ch + 1) * 512],
                )
                te = nc.vector if ch % 2 == 0 else nc.gpsimd
                nc.vector.tensor_scalar(
                    oh[:, ch * 512 : (ch + 1) * 512], br_ps[:],
                    iota_f[:, 0:1], None, op0=OP.is_equal,
                )

            # ---------------- per-tile one-hot gather + epilogue -----------
            o_all = cpool.tile([128, NT_I * F], F32)
            o3 = o_all[:].rearrange("p (t c) -> p t c", c=F)
            out_view = out_d.ap().rearrange("(a p) f -> p a f", p=128)
            for it in range(NT_I):
                a_ps = ppool.tile([128, F + 1], F32, tag="aps", bufs=2)
                d_ps = ppool.tile([128, F + 1], F32, tag="dps", bufs=2)
                nc.tensor.matmul(
                    a_ps[:], oh[:, it * 128 : (it + 1) * 128], gvT[:]
                )
                i2 = nc.tensor.matmul(
                    d_ps[:], oh[:, it * 128 : (it + 1) * 128], gdT[:]
                )
                i2.ins.ldweights = False
                a_sb = wpool.tile([128, F + 1], F32, tag="asb", bufs=3)
                nc.scalar.copy(a_sb[:], a_ps[:])
                num = wpool.tile([128, F], F32, tag="num", bufs=3)
                nc.vector.scalar_tensor_tensor(
                    num[:], d_ps[:, 0:F], negr_col[:, it : it + 1],
                    a_sb[:, 0:F], op0=OP.mult, op1=OP.add,
                )
                den = wpool.tile([128, 1], F32, tag="den", bufs=3)
                nc.vector.scalar_tensor_tensor(
                    den[:], d_ps[:, F : F + 1], negr_col[:, it : it + 1],
                    a_sb[:, F : F + 1], op0=OP.mult, op1=OP.add,
                )
                rec = wpool.tile([128, 1], F32, tag="rec", bufs=3)
                nc.vector.reciprocal(rec[:], den[:])
                nc.vector.scalar_tensor_tensor(
                    o3[:, it, :], num[:], rec[:, 0:1], bias_rep[:],
                    op0=OP.mult, op1=OP.add,
                )
                if it % 8 == 7:
                    grp = it // 8
                    nc.sync.dma_start(
                        out_view[:, grp * 8 : (grp + 1) * 8, :],
                        o_all[:, grp * 8 * F : (grp + 1) * 8 * F],
                    )

    nc.compile()
    return nc


_NC_CACHE = None


def _get_nc():
    global _NC_CACHE
    if _NC_CACHE is None:
        _NC_CACHE = _build_kernel_module()
    return _NC_CACHE


def _make_in_maps(h, w, a_src, a_dst, bias):
    h = np.ascontiguousarray(np.asarray(h, dtype=np.float32))
    w = np.asarray(w, dtype=np.float32)
    a_src = np.asarray(a_src, dtype=np.float32)
    a_dst = np.asarray(a_dst, dtype=np.float32)
    bias = np.asarray(bias, dtype=np.float32).reshape(1, F)
    in_maps = []
    for c in range(8):
        head, half = c // 2, c % 2
        aa = np.ascontiguousarray(
            np.concatenate([a_src[head], a_dst[head]], axis=1)
        )
        # rotate rows so this core's query half is rows 0..HALF-1
        hrot = np.ascontiguousarray(
            np.concatenate([h[half * HALF :], h[: half * HALF]])
        )
        in_maps.append(
            {
                "hfull": hrot,
                "w": np.ascontiguousarray(w[head]),
                "aa": aa,
                "bias": bias,
            }
        )
    return in_maps


def _run(h, w, a_src, a_dst, bias, trace=False, **trace_kwargs):
    nc = _get_nc()
    in_maps = _make_in_maps(h, w, a_src, a_dst, bias)
    res = run_bass_kernel_spmd(
        nc, in_maps, core_ids=list(range(8)), trace=trace, **trace_kwargs
    )
    out = np.zeros((BS, NH * F), dtype=np.float32)
    for c in range(8):
        head, half = c // 2, c % 2
        out[half * HALF : (half + 1) * HALF, head * F : (head + 1) * F] = res.results[
            c
        ]["out"]
    return out, res


def kernel(h, w, a_src, a_dst, bias):
    out, _ = _run(h, w, a_src, a_dst, bias, trace=False)
    return out


# revision 22
# speedup vs baseline: 3.8092x; 1.0587x over previous
"""Multi-head graph-attention (GAT) kernel for Trainium2, 8 NeuronCores.

Reference computation (per head):
    h_prime = h @ w[head]                       # [8192, 64]
    s = h_prime @ a_src[head],  d = h_prime @ a_dst[head]
    attn = softmax_j(leaky_relu(s_i + d_j, 0.2))
    out  = attn @ h_prime + bias                # -> [8192, 4*64]

Key identity: with exp monotone,
    exp(lrelu(s_i + d_j)) = e^{s_i} e^{d_j}           if s_i + d_j >= 0
                          = e^{0.2 s_i} e^{0.2 d_j}   otherwise
The mask sets {j : d_j >= -s_i} are NESTED across i, so the whole O(n^2)
attention contraction is a 1-D step function of t = -s_i:
    g(t) = sum_{j: d_j >= t} [v_j h'_j | v_j],   v = e^d   (and gq with q)
Evaluate g on a fixed grid of B=128 thresholds t_b (one fp16 matmul with
only B moving columns per 128-row j-tile), then each query i picks its
column b_i = clamp((-s_i - LO)/G) via a one-hot matmul that lands the
result directly in query-partition layout.  Grid rounding only
misclassifies js with |s_i + d_j| <= G, where exp(lrelu(x)) is
continuous, so the error is O(G^2) ~ 3e-4.  Everything else is exact:
    num_i / e^{s_i} = A(b_i) - r_i (C(b_i) - Sq),   r_i = e^{-0.8 s_i}
    den_i / e^{s_i} = a(b_i) - r_i (c(b_i) - sq)
with A|a = g_v columns, C|c = g_q columns, Sq|sq the full q-sums
(= g_q at the leftmost grid point).

Sharding: 8 cores = 4 heads x 2 query-halves. Each core gets the full h
(rows rotated so its query half is rows 0..4095 - j-side sums are order
invariant), computes the full j-side tables, and evaluates its 4096
queries. No collectives.
"""

import numpy as np

import concourse.bass as bass
import concourse.tile as tile
from concourse import bacc, mybir
from concourse.bass_utils import run_bass_kernel_spmd
from concourse.masks import make_identity

F32 = mybir.dt.float32
F32R = mybir.dt.float32r
BF16 = mybir.dt.bfloat16
FP16 = mybir.dt.float16
I32 = mybir.dt.int32
AF = mybir.ActivationFunctionType
OP = mybir.AluOpType

BS = 8192          # nodes
F = 64             # f_in == f_out
NH = 4             # heads
HALF = BS // 2     # queries per core
NT_J = BS // 128   # 64 j tiles
NT_I = HALF // 128 # 32 query tiles
ALPHA = 0.2

BGRID = 128        # grid points (one PE tile)
LO = -8.0          # grid range [LO, -LO)
G = (-2.0 * LO) / BGRID   # 1/8
INVG = 1.0 / G

AUGW = 2 * (F + 1)  # 130: [Hv|v | Hq|q] stationary width


def _build_kernel_module():
    nc = bacc.Bacc("TRN2", target_bir_lowering=False, debug=False)

    hfull_d = nc.dram_tensor("hfull", [BS, F], F32, kind="ExternalInput")
    w_d = nc.dram_tensor("w", [F, F], F32, kind="ExternalInput")
    aa_d = nc.dram_tensor("aa", [F, 2], F32, kind="ExternalInput")
    bias_d = nc.dram_tensor("bias", [1, F], F32, kind="ExternalInput")
    out_d = nc.dram_tensor("out", [HALF, F], F32, kind="ExternalOutput")

    with tile.TileContext(nc) as tc:
        with (
            tc.tile_pool(name="const", bufs=1) as cpool,
            tc.tile_pool(name="work", bufs=3) as wpool,
            tc.tile_pool(name="dscr", bufs=1, space="DRAM") as dpool,
            tc.tile_pool(name="psum", bufs=2, space="PSUM") as ppool,
        ):
            # ---------------- constants ----------------
            identity = cpool.tile([128, 128], F32)
            make_identity(nc, identity[:])
            ones = cpool.tile([128, 128], F32)
            nc.gpsimd.memset(ones[:], 1.0)
            iota_col = cpool.tile([128, 1], I32)
            nc.gpsimd.iota(iota_col[:], [[1, 1]], base=0, channel_multiplier=1)
            iota_f = cpool.tile([128, 1], F32)
            nc.vector.tensor_copy(iota_f[:], iota_col[:])

            # ---------------- tiny weight prep ----------------
            w_sb = cpool.tile([F, F], F32)
            nc.sync.dma_start(w_sb[:], w_d.ap())
            aa_sb = cpool.tile([F, 2], F32)
            nc.sync.dma_start(aa_sb[:], aa_d.ap())
            bias_sb = cpool.tile([1, F], F32)
            nc.sync.dma_start(bias_sb[:], bias_d.ap())

            wT_ps = ppool.tile([F, F], F32, tag="mix")
            nc.tensor.transpose(wT_ps[:], w_sb[:], identity[0:F, 0:F])
            wT_sb = cpool.tile([F, F], F32)
            nc.scalar.copy(wT_sb[:], wT_ps[:])

            # ws = w @ [a_src | a_dst]  -> [64, 2]
            ws_ps = ppool.tile([F, 2], F32, tag="mix")
            nc.tensor.matmul(ws_ps[:], wT_sb[:], aa_sb[:])
            ws_sb = cpool.tile([F, 2], F32)
            nc.scalar.copy(ws_sb[:], ws_ps[:])

            # w_aug = [w | w@a_dst | w@a_src]: h @ w_aug -> [h' | d | s]
            # replicated on partitions 64:128 for odd-tile stationaries
            w_top = cpool.tile([F, F + 2], F32R)
            nc.scalar.copy(w_top[:, 0:F], w_sb[:])
            nc.scalar.copy(w_top[:, F : F + 1], ws_sb[:, 1:2])
            nc.scalar.copy(w_top[:, F + 1 : F + 2], ws_sb[:, 0:1])
            dbl_id = cpool.tile([F, 128], F32R)
            nc.scalar.copy(dbl_id[:, 0:F], identity[0:F, 0:F])
            nc.vector.tensor_copy(dbl_id[:, F:128], identity[0:F, 0:F])
            wa_ps = ppool.tile([128, F + 2], F32, tag="mix")
            nc.tensor.matmul(wa_ps[:], dbl_id[:], w_top[:])
            w_aug = cpool.tile([128, F + 2], F32R)
            nc.scalar.copy(w_aug[:], wa_ps[:])

            # bias broadcast to all partitions
            biasb_ps = ppool.tile([128, F], F32, tag="mix")
            nc.tensor.matmul(biasb_ps[:], ones[0:1, 0:128], bias_sb[:])
            bias_rep = cpool.tile([128, F], F32)
            nc.scalar.copy(bias_rep[:], biasb_ps[:])

            # grid thresholds replicated across partitions, bf16-exact
            tg_f = cpool.tile([1, BGRID], F32)
            nc.gpsimd.iota(
                tg_f[:], [[1, BGRID]], base=0, channel_multiplier=0,
                allow_small_or_imprecise_dtypes=True,
            )
            tgrid = cpool.tile([1, BGRID], F32)
            nc.vector.tensor_scalar(
                tgrid[:], tg_f[:], G, LO, op0=OP.mult, op1=OP.add
            )
            trow_ps = ppool.tile([128, BGRID], F32, tag="mix")
            nc.tensor.matmul(trow_ps[:], ones[0:1, 0:128], tgrid[:])
            trow_b = cpool.tile([128, BGRID], BF16)
            nc.scalar.copy(trow_b[:], trow_ps[:])

            # ---------------- h load, h^T (paired), h' (+d +s) -------------
            # hT2 chunk p holds transposed tiles 2p (partitions 0:64) and
            # 2p+1 (partitions 64:128).
            hT2 = cpool.tile([128, (NT_J // 2) * 128], F32R)
            hprB = cpool.tile([128, NT_J * F], BF16)   # h' in bf16
            hprB3 = hprB[:].rearrange("p (t c) -> p t c", c=F)
            ds_col = cpool.tile([128, NT_J * 2], F32)  # [d | s] per tile
            ds3 = ds_col[:].rearrange("p (t c) -> p t c", c=2)
            hf_view = hfull_d.ap().rearrange("(a p) f -> p a f", p=128)
            for blk in range(NT_J // 8):
                ldb = wpool.tile([128, 8 * F], F32, tag="hloadb", bufs=3)
                nc.sync.dma_start(
                    ldb[:], hf_view[:, blk * 8 : (blk + 1) * 8, :]
                )
                for k in range(4):
                    pr = blk * 4 + k  # pair index
                    tr = ppool.tile([128, 128], F32, tag="mix")
                    nc.tensor.transpose(
                        tr[:],
                        ldb[:, k * 128 : (k + 1) * 128],
                        identity[:],
                    )
                    if k % 2 == 0:
                        nc.scalar.copy(hT2[:, pr * 128 : (pr + 1) * 128], tr[:])
                    else:
                        nc.vector.tensor_copy(
                            hT2[:, pr * 128 : (pr + 1) * 128], tr[:]
                        )
            for jt in range(NT_J):
                pr, par = jt // 2, (jt % 2) * F
                hp_ps = ppool.tile([128, F + 2], F32, tag="mix")
                nc.tensor.matmul(
                    hp_ps[:],
                    hT2[par : par + F, pr * 128 : (pr + 1) * 128],
                    w_aug[par : par + F, :],
                )
                if jt % 2 == 0:
                    nc.scalar.copy(hprB3[:, jt, :], hp_ps[:, 0:F])
                    nc.vector.tensor_copy(ds3[:, jt, :], hp_ps[:, F : F + 2])
                else:
                    nc.vector.tensor_copy(hprB3[:, jt, :], hp_ps[:, 0:F])
                    nc.scalar.copy(ds3[:, jt, :], hp_ps[:, F : F + 2])

            d_all = ds3[:, :, 0]            # [128, NT_J] strided
            s_half = ds3[:, 0:NT_I, 1]      # queries = rows 0..4095

            v_col = cpool.tile([128, NT_J], F32)
            q_col = cpool.tile([128, NT_J], F32)

            # ---------------- query-side prep ----------------
            negr_col = cpool.tile([128, NT_I], F32)
            nc.scalar.activation(negr_col[:], s_half, AF.Exp, scale=-(1.0 - ALPHA))
            nc.vector.tensor_scalar_mul(negr_col[:], negr_col[:], -1.0)

            b_f = wpool.tile([128, NT_I], F32, tag="bf", bufs=1)
            nc.vector.tensor_scalar(
                b_f[:], s_half, -INVG, -LO * INVG, op0=OP.mult, op1=OP.add
            )
            b_rnd = wpool.tile([128, NT_I], F32, tag="brnd", bufs=1)
            nc.vector.tensor_scalar(
                b_rnd[:], b_f[:], 8388608.0, 8388608.0,
                op0=OP.add, op1=OP.subtract,
            )
            b_cl = wpool.tile([128, NT_I], BF16, tag="bcl", bufs=1)
            nc.vector.tensor_scalar(
                b_cl[:], b_rnd[:], 0.0, float(BGRID - 1), op0=OP.max, op1=OP.min
            )
            # roundtrip through DRAM to get b as a single row [1, 4096]
            b_dram = dpool.tile([HALF], BF16, name="bscr")
            nc.sync.dma_start(
                b_dram[:].rearrange("(c q) -> q c", q=128), b_cl[:]
            )
            b_row = cpool.tile([1, HALF], BF16)
            ones_bf = cpool.tile([1, 128], BF16)
            nc.vector.tensor_copy(ones_bf[:], ones[0:1, 0:128])
            nc.sync.dma_start(b_row[:], b_dram[:].rearrange("(a n) -> a n", a=1))

            # ------- fused: exps + ST build + mask + grid matmul -------
            bankV = ppool.tile([F + 1, BGRID], F32, tag="bankV", bufs=1)
            bankQ = ppool.tile([F + 1, BGRID], F32, tag="bankQ", bufs=1)
            GRP = 16
            for jt in range(NT_J):
                if jt % GRP == 0:
                    gs = slice(jt, jt + GRP)
                    nc.scalar.activation(v_col[:, gs], d_all[:, gs], AF.Exp)
                    nc.scalar.activation(
                        q_col[:, gs], d_all[:, gs], AF.Exp, scale=ALPHA
                    )
                st_t = wpool.tile([128, AUGW], FP16, tag="stt", bufs=4)
                if jt % 4 == 3:
                    nc.scalar.activation(
                        st_t[:, 0:F], hprB3[:, jt, :], AF.Identity,
                        scale=v_col[:, jt : jt + 1],
                    )
                    nc.scalar.activation(
                        st_t[:, F + 1 : AUGW - 1], hprB3[:, jt, :],
                        AF.Identity, scale=q_col[:, jt : jt + 1],
                    )
                else:
                    nc.vector.tensor_scalar_mul(
                        st_t[:, 0:F], hprB3[:, jt, :], v_col[:, jt : jt + 1]
                    )
                    nc.vector.tensor_scalar_mul(
                        st_t[:, F + 1 : AUGW - 1], hprB3[:, jt, :],
                        q_col[:, jt : jt + 1],
                    )
                nc.vector.tensor_copy(
                    st_t[:, F : F + 1], v_col[:, jt : jt + 1]
                )
                nc.vector.tensor_copy(
                    st_t[:, AUGW - 1 : AUGW], q_col[:, jt : jt + 1]
                )
                mw = wpool.tile([128, BGRID], FP16, tag="mask", bufs=4)
                nc.vector.tensor_scalar(
                    mw[:], trow_b[:], ds3[:, jt, 0:1], None, op0=OP.is_le
                )
                st, sp = (jt == 0), (jt == NT_J - 1)
                nc.tensor.matmul(
                    bankV[:], st_t[:, 0 : F + 1], mw[:], start=st, stop=sp
                )
                nc.tensor.matmul(
                    bankQ[:], st_t[:, F + 1 : AUGW], mw[:], start=st, stop=sp
                )

            # tables: A = g_v, D = g_q - Sq  (Sq = g_q[:, 0]); transpose to
            # [grid-part, comp] fp16 for the one-hot matmuls
            sq_col = cpool.tile([F + 1, 1], F32)
            nc.vector.tensor_copy(sq_col[:], bankQ[:, 0:1])
            A_sb = cpool.tile([F + 1, BGRID], F32)
            nc.scalar.copy(A_sb[:], bankV[:])
            D_sb = cpool.tile([F + 1, BGRID], F32)
            nc.vector.tensor_scalar(
                D_sb[:], bankQ[:], sq_col[:, 0:1], None, op0=OP.subtract
            )
            gvT_ps = ppool.tile([BGRID, F + 1], F32, tag="mix")
            nc.tensor.transpose(
                gvT_ps[:], A_sb[:],
                identity[0 : F + 1, 0 : F + 1],
            )
            gvd = cpool.tile([BGRID, AUGW], FP16)
            nc.scalar.copy(gvd[:, 0 : F + 1], gvT_ps[:])
            gdT_ps = ppool.tile([BGRID, F + 1], F32, tag="mix")
            nc.tensor.transpose(
                gdT_ps[:], D_sb[:],
                identity[0 : F + 1, 0 : F + 1],
            )
            nc.vector.tensor_copy(gdT_ps_sb_dummy := gvd[:, F + 1 : AUGW], gdT_ps[:])

            # one-hot of query buckets: oh[b, i] = (b_i == b), fp16
            oh = cpool.tile([128, HALF], FP16)
            for ch in range(8):
                br_ps = ppool.tile([128, 512], F32, tag="mix")
                nc.tensor.matmul(
                    br_ps[:],
                    ones_bf[:],
                    b_row[:, ch * 512 : (ch + 1) * 512],
                )
                te = nc.vector if ch % 2 == 0 else nc.gpsimd
                nc.vector.tensor_scalar(
                    oh[:, ch * 512 : (ch + 1) * 512], br_ps[:],
                    iota_f[:, 0:1], None, op0=OP.is_equal,
                )

            # ---------------- per-tile one-hot gather + epilogue -----------
            o_all = cpool.tile([128, NT_I * F], F32)
            o3 = o_all[:].rearrange("p (t c) -> p t c", c=F)
            out_view = out_d.ap().rearrange("(a p) f -> p a f", p=128)
            for it in range(NT_I):
                ad_ps = ppool.tile([128, AUGW], F32, tag="adps", bufs=3)
                nc.tensor.matmul(
                    ad_ps[:], oh[:, it * 128 : (it + 1) * 128], gvd[:]
                )
                a_sb = wpool.tile([128, F + 1], F32, tag="asb", bufs=3)
                nc.scalar.copy(a_sb[:], ad_ps[:, 0 : F + 1])
                num = wpool.tile([128, F], F32, tag="num", bufs=3)
                nc.vector.scalar_tensor_tensor(
                    num[:], ad_ps[:, F + 1 : AUGW - 1],
                    negr_col[:, it : it + 1], a_sb[:, 0:F],
                    op0=OP.mult, op1=OP.add,
                )
                den = wpool.tile([128, 1], F32, tag="den", bufs=3)
                nc.scalar.activation(
                    den[:], ad_ps[:, AUGW - 1 : AUGW], AF.Identity,
                    scale=negr_col[:, it : it + 1],
                    bias=a_sb[:, F : F + 1],
                )
                rec = wpool.tile([128, 1], F32, tag="rec", bufs=3)
                nc.vector.reciprocal(rec[:], den[:])
                nc.vector.scalar_tensor_tensor(
                    o3[:, it, :], num[:], rec[:, 0:1], bias_rep[:],
                    op0=OP.mult, op1=OP.add,
                )
                if it % 8 == 7:
                    grp = it // 8
                    nc.sync.dma_start(
                        out_view[:, grp * 8 : (grp + 1) * 8, :],
                        o_all[:, grp * 8 * F : (grp + 1) * 8 * F],
                    )

    nc.compile()
    return nc


_NC_CACHE = None


def _get_nc():
    global _NC_CACHE
    if _NC_CACHE is None:
        _NC_CACHE = _build_kernel_module()
    return _NC_CACHE


def _make_in_maps(h, w, a_src, a_dst, bias):
    h = np.ascontiguousarray(np.asarray(h, dtype=np.float32))
    w = np.asarray(w, dtype=np.float32)
    a_src = np.asarray(a_src, dtype=np.float32)
    a_dst = np.asarray(a_dst, dtype=np.float32)
    bias = np.asarray(bias, dtype=np.float32).reshape(1, F)
    in_maps = []
    for c in range(8):
        head, half = c // 2, c % 2
        aa = np.ascontiguousarray(
            np.concatenate([a_src[head], a_dst[head]], axis=1)
        )
        # rotate rows so this core's query half is rows 0..HALF-1
        hrot = np.ascontiguousarray(
            np.concatenate([h[half * HALF :], h[: half * HALF]])
        )
        in_maps.append(
            {
                "hfull": hrot,
                "w": np.ascontiguousarray(w[head]),
                "aa": aa,
                "bias": bias,
            }
        )
    return in_maps


def _run(h, w, a_src, a_dst, bias, trace=False, **trace_kwargs):
    nc = _get_nc()
    in_maps = _make_in_maps(h, w, a_src, a_dst, bias)
    res = run_bass_kernel_spmd(
        nc, in_maps, core_ids=list(range(8)), trace=trace, **trace_kwargs
    )
    out = np.zeros((BS, NH * F), dtype=np.float32)
    for c in range(8):
        head, half = c // 2, c % 2
        out[half * HALF : (half + 1) * HALF, head * F : (head + 1) * F] = res.results[
            c
        ]["out"]
    return out, res


def kernel(h, w, a_src, a_dst, bias):
    out, _ = _run(h, w, a_src, a_dst, bias, trace=False)
    return out
